# revision 19
# baseline (speedup 1.0000x reference)
"""Trainium2 Bass kernel for nn_L1OutUB (L1-out upper bound contrastive loss).

Math: the reference builds a [B,B,B] tensor `inpt[a,i,j] = all_probs[i,j] +
(-20 if a==i else 0)` and logsumexps over `a`.  That logsumexp is exactly
`all_probs[i,j] + log(B-1+e^-20)`, so

    result = mean(positive) - mean(all_probs) - log1p(e^-20 / (B-1))

`sum_j all_probs[i,j]` collapses onto per-column moments of y, and the
-0.5*logvar terms cancel between positive and negative.  Expanding the
positive-branch square as well, every remaining term is a contraction of
per-core row sums:

    A[d]  = sum_i iv[i,d]          C[d]  = sum_i mu[i,d] iv[i,d]
    D     = sum_{i,d} mu^2 iv      U1    = sum_{i,d} mu y iv   (matched y)
    U2    = sum_{i,d} y^2 iv       S2[d] = sum_j y[j,d]^2      M1[d] = sum_j y[j,d]

    P     = D - 2 U1 + U2          (positive-branch quadratic)
    result = -(P/2B) + (1/2B^2)(S2.A - 2 M1.C + B D) - log1p(e^-20/(B-1))

Sharding: rows of x/y across 8 cores (64 rows each); each core emits its
partial vectors [128, 7] = (A, C, Dv, U1v, U2v, S2, M1); the host sums the
8 partials and does three dot products (the "all-reduce").

Device-side structure per core (layout [d=128 partitions, r=64 free]):
  - raw Bass (no TileContext): one basic block, hand-placed semaphores.
    DMA issues are the first sequencer instructions, so transfers start as
    early as the runtime allows.
  - host pre-transposes x (xT chunks, fp16) and y (yT), packs both MLPs'
    layer-1 weights into one dense fp16 [128,18] lhsT per chunk whose two
    zero columns, via relu(0 + bias=1), manufacture ones-rows that fold
    the layer-2 biases into the matmuls.  fp16 operands keep each matmul
    a single PE pass and halve the DMA bytes; b1 rides the weight blob.
  - PE: 6 accumulating L1 matmuls (second x half first - it lands first)
    -> relu -> 2 L2 matmuls (fp16 w2 lhsT, zero-padded to base partition
    0); z's matmul goes first so tanh starts while mu's still runs.
  - Scalar: relu, tanh(scale=-1), exp (iv = exp(-tanh(z)); logvar cancels).
  - DVE: iv + 4 products live in one [128, 5*64] tile; a single segmented
    tensor_reduce emits A, C, Dv, U1v, U2v at once.  y moments off-path.
  - No transposes, no GpSimd, no collectives; one [128,7] output DMA.
"""

import numpy as np

import concourse.bacc as bacc
from concourse import mybir

F32 = mybir.dt.float32
FP16 = mybir.dt.float16
AF = mybir.ActivationFunctionType
ALU = mybir.AluOpType
AX = mybir.AxisListType

B, X_DIM, Y_DIM, HID = 512, 768, 128, 8
N_CORES = 8
R = B // N_CORES          # rows per core = 64
XC = X_DIM // 128         # x feature chunks = 6
WCOL = 18                 # packed L1 lhsT cols: 0:8 mu, 8 zero, 9:17 lv, 17 zero

_CACHE = {}


def _build():
    nc = bacc.Bacc("TRN2", target_bir_lowering=False, debug=False,
                   num_devices=N_CORES)

    # wp: 6 chunks of w1p [128,18] + col 108 = b1vec
    #     (rows 0:8 b1_mu, row 8 = 1.0, rows 9:17 b1_lv, row 17 = 1.0)
    wp_d = nc.dram_tensor("wp", [128, XC * WCOL + 1], FP16,
                          kind="ExternalInput")
    x1_d = nc.dram_tensor("x1", [128, 3 * R], FP16, kind="ExternalInput")
    # x2: xT chunks 3..5 ++ yT in fp16 (cols 192:256)
    x2_d = nc.dram_tensor("x2", [128, 4 * R], FP16, kind="ExternalInput")
    # w2: cols 0:128 = mu block (rows 0:8 w2_mu, row 8 b2_mu, rows 9:18 zero)
    #     cols 128:256 = lv block (rows 0:9 zero, rows 9:17 w2_lv, row 17 b2_lv)
    w2_d = nc.dram_tensor("w2", [WCOL, 256], FP16, kind="ExternalInput")
    # out columns: A, C, Dv, U1v, U2v, S2, M1
    out_d = nc.dram_tensor("out", [128, 7], F32, kind="ExternalOutput")

    from contextlib import ExitStack
    with ExitStack() as stack:
        e = stack.enter_context
        wp_s = e(nc.sbuf_tensor([128, XC * WCOL + 1], FP16))
        x1_s = e(nc.sbuf_tensor([128, 3 * R], FP16))
        x2_s = e(nc.sbuf_tensor([128, 4 * R], FP16))
        w2_s = e(nc.sbuf_tensor([WCOL, 256], FP16))
        yt_s = e(nc.sbuf_tensor([128, R], F32))   # f32 copy of yT
        hb_s = e(nc.sbuf_tensor([WCOL, R], FP16))
        th_s = e(nc.sbuf_tensor([128, R], F32))
        prods = e(nc.sbuf_tensor([128, 5, R], F32))
        ysq_s = e(nc.sbuf_tensor([128, R], F32))
        outv = e(nc.sbuf_tensor([128, 7], F32))
        ps1 = e(nc.psum_tensor([WCOL, R], F32))
        ps2l = e(nc.psum_tensor([128, R], F32))
        ps2m = e(nc.psum_tensor([128, R], F32))
        wsem = e(nc.semaphore())    # wp landed
        x1sem = e(nc.semaphore())
        x2sem = e(nc.semaphore())
        w2sem = e(nc.semaphore())
        gsem = e(nc.semaphore())    # gpsimd: yt converted
        tsem = e(nc.semaphore())    # tensor progress: 1=L1, 2=ps2l, 3=ps2m
        ssem = e(nc.semaphore())    # scalar progress: 1=relu, 2=exp
        vsem = e(nc.semaphore())    # vector progress: 1=y-moments, 2=big reduce
        osem = e(nc.semaphore())    # out DMA completion (not awaited)
        vq = e(nc.semaphore())      # vector same-engine chain
        sq = e(nc.semaphore())      # scalar same-engine chain
        tq = e(nc.semaphore())      # tensor same-engine chain
        block = e(nc.Block())
        yT = yt_s[:]
        iv = prods[:, 0, :]
        t1 = prods[:, 1, :]

        @block.sync
        def _(sync):
            sync.dma_start(out=wp_s[:], in_=wp_d[:]).then_inc(wsem, 16)
            sync.dma_start(out=x1_s[:], in_=x1_d[:]).then_inc(x1sem, 16)
            sync.wait_ge(vsem, 2)
            sync.dma_start(out=out_d[:], in_=outv[:]).then_inc(osem, 16)

        @block.scalar
        def _(scalar):
            scalar.dma_start(out=x2_s[:], in_=x2_d[:]).then_inc(x2sem, 16)
            scalar.dma_start(out=w2_s[:], in_=w2_d[:]).then_inc(w2sem, 16)
            scalar.wait_ge(tsem, 1)
            scalar.wait_ge(wsem, 16)
            scalar.activation(out=hb_s[:], in_=ps1[:], func=AF.Relu,
                              bias=wp_s[0:WCOL, XC * WCOL:XC * WCOL + 1]
                              ).then_inc(ssem, 1)
            scalar.wait_ge(tsem, 2)
            scalar.activation(out=th_s[:], in_=ps2l[:], func=AF.Tanh,
                              scale=-1.0).then_inc(sq, 1)
            scalar.wait_ge(sq, 1)
            scalar.activation(out=iv, in_=th_s[:], func=AF.Exp
                              ).then_inc(ssem, 1)

        @block.tensor
        def _(tensor):
            tensor.wait_ge(wsem, 16)
            tensor.wait_ge(x2sem, 16)
            order = [3, 4, 5, 0, 1, 2]
            for i, k in enumerate(order):
                if k == 0:
                    tensor.wait_ge(x1sem, 16)
                src = x2_s if k >= 3 else x1_s
                j = (k % 3) * R
                inst = tensor.matmul(ps1[:],
                                     wp_s[:, k * WCOL:(k + 1) * WCOL],
                                     src[:, j:j + R],
                                     start=(i == 0), stop=(i == XC - 1))
                if i == XC - 1:
                    inst.then_inc(tsem, 1)
            tensor.wait_ge(ssem, 1)
            tensor.wait_ge(w2sem, 16)
            tensor.matmul(ps2l[:], w2_s[:, 128:256], hb_s[:],
                          start=True, stop=True).then_inc(tsem, 1)
            tensor.matmul(ps2m[:], w2_s[:, 0:128], hb_s[:],
                          start=True, stop=True).then_inc(tsem, 1)

        @block.gpsimd
        def _(gpsimd):
            gpsimd.wait_ge(x2sem, 16)
            gpsimd.tensor_copy(out=yt_s[:], in_=x2_s[:, 3 * R:4 * R]
                               ).then_inc(gsem, 1)

        @block.vector
        def _(vector):
            vector.wait_ge(gsem, 1)
            vector.tensor_mul(ysq_s[:], yT, yT).then_inc(vq, 1)
            vector.wait_ge(vq, 1)
            vector.tensor_reduce(out=outv[:, 5:6], in_=ysq_s[:],
                                 axis=AX.X, op=ALU.add)
            vector.tensor_reduce(out=outv[:, 6:7], in_=yT,
                                 axis=AX.X, op=ALU.add).then_inc(vsem, 1)
            vector.wait_ge(ssem, 2)
            vector.wait_ge(tsem, 3)
            vector.tensor_mul(t1, ps2m[:], iv).then_inc(vq, 1)
            vector.wait_ge(vq, 2)
            vector.tensor_mul(prods[:, 2, :], t1, ps2m[:]).then_inc(vq, 1)
            vector.tensor_mul(prods[:, 3, :], t1, yT).then_inc(vq, 1)
            vector.tensor_mul(prods[:, 4, :], ysq_s[:], iv).then_inc(vq, 1)
            vector.wait_ge(vq, 5)
            vector.tensor_reduce(out=outv[:, 0:5], in_=prods[:],
                                 axis=AX.X, op=ALU.add).then_inc(vsem, 1)

    nc.compile()
    return nc


def _get_nc():
    if "nc" not in _CACHE:
        _CACHE["nc"] = _build()
    return _CACHE["nc"]


def _pack(x_samples, y_samples, w1_mu, b1_mu, w2_mu, b2_mu,
          w1_lv, b1_lv, w2_lv, b2_lv):
    f = np.float32
    w1m = np.asarray(w1_mu, f).reshape(XC, 128, HID)
    w1l = np.asarray(w1_lv, f).reshape(XC, 128, HID)
    wp = np.zeros((128, XC * WCOL + 1), f)
    for k in range(XC):
        wp[:, k * WCOL:k * WCOL + 8] = w1m[k]
        wp[:, k * WCOL + 9:k * WCOL + 17] = w1l[k]
    wp[0:8, XC * WCOL] = np.asarray(b1_mu, f)
    wp[8, XC * WCOL] = 1.0
    wp[9:17, XC * WCOL] = np.asarray(b1_lv, f)
    wp[17, XC * WCOL] = 1.0
    wp16 = wp.astype(np.float16)

    w2 = np.zeros((WCOL, 256), f)
    w2[0:8, 0:128] = np.asarray(w2_mu, f)
    w2[8, 0:128] = np.asarray(b2_mu, f)
    w2[9:17, 128:256] = np.asarray(w2_lv, f)
    w2[17, 128:256] = np.asarray(b2_lv, f)
    w2 = w2.astype(np.float16)

    x = np.asarray(x_samples, f)
    y = np.asarray(y_samples, f)
    in_maps = []
    for c in range(N_CORES):
        xs = x[c * R:(c + 1) * R]                       # [64, 768]
        xT = xs.reshape(R, XC, 128).transpose(1, 2, 0).astype(np.float16)
        x1 = np.ascontiguousarray(
            xT[0:3].transpose(1, 0, 2).reshape(128, 3 * R))
        x2 = np.empty((128, 4 * R), np.float16)
        x2[:, 0:3 * R] = xT[3:6].transpose(1, 0, 2).reshape(128, 3 * R)
        x2[:, 3 * R:4 * R] = y[c * R:(c + 1) * R].T.astype(np.float16)
        in_maps.append({"wp": wp16, "x1": x1, "x2": np.ascontiguousarray(x2),
                        "w2": w2})
    return in_maps


def kernel(x_samples, y_samples, w1_mu, b1_mu, w2_mu, b2_mu,
           w1_lv, b1_lv, w2_lv, b2_lv, **profile_kwargs):
    from concourse import bass_utils

    in_maps = _pack(x_samples, y_samples, w1_mu, b1_mu, w2_mu, b2_mu,
                    w1_lv, b1_lv, w2_lv, b2_lv)
    nc = _get_nc()
    res = bass_utils.run_bass_kernel_spmd(
        nc, in_maps, core_ids=list(range(N_CORES)), **profile_kwargs
    )
    acc = np.zeros((128, 7), np.float64)
    for m in res.results:
        acc += m["out"].astype(np.float64)
    A, C, Dv, U1v, U2v, S2, M1 = (acc[:, j] for j in range(7))
    D = Dv.sum()
    P = D - 2.0 * U1v.sum() + U2v.sum()
    neg = (S2 @ A - 2.0 * (M1 @ C) + B * D) / (2.0 * B * B)
    total = -P / (2.0 * B) + neg - np.log1p(np.exp(-20.0) / (B - 1.0))
    out = np.array(total, dtype=np.float32)
    if profile_kwargs:
        return out, res
    return out


# revision 20
# speedup vs baseline: 1.1590x; 1.1590x over previous
"""Trainium2 Bass kernel for nn_L1OutUB (L1-out upper bound contrastive loss).

Math: the reference builds a [B,B,B] tensor `inpt[a,i,j] = all_probs[i,j] +
(-20 if a==i else 0)` and logsumexps over `a`.  That logsumexp is exactly
`all_probs[i,j] + log(B-1+e^-20)`, so

    result = mean(positive) - mean(all_probs) - log1p(e^-20 / (B-1))

`sum_j all_probs[i,j]` collapses onto per-column moments of y, and the
-0.5*logvar terms cancel between positive and negative.  Expanding the
positive-branch square as well, every remaining term is a contraction of
per-core row sums:

    A[d]  = sum_i iv[i,d]          C[d]  = sum_i mu[i,d] iv[i,d]
    D     = sum_{i,d} mu^2 iv      U1    = sum_{i,d} mu y iv   (matched y)
    U2    = sum_{i,d} y^2 iv       S2[d] = sum_j y[j,d]^2      M1[d] = sum_j y[j,d]

    P     = D - 2 U1 + U2          (positive-branch quadratic)
    result = -(P/2B) + (1/2B^2)(S2.A - 2 M1.C + B D) - log1p(e^-20/(B-1))

Sharding: rows of x/y across 8 cores (64 rows each); each core emits its
partial vectors [128, 7] = (A, C, Dv, U1v, U2v, S2, M1); the host sums the
8 partials and does three dot products (the "all-reduce").

Device-side structure per core (layout [d=128 partitions, r=64 free]):
  - raw Bass (no TileContext): one basic block, hand-placed semaphores.
  - ONE fp16 input mega-blob [128, 557] = [w1 packed chunks + b1vec | xT
    chunks | yT] so the input DMA moves 128 long rows (descriptor count,
    not bytes, dominates landing time); w2 rides the other ring (18 rows).
  - host pre-transposes x and y; both MLPs' layer-1 weights pack into one
    dense fp16 [128,18] lhsT per chunk whose two zero columns, via
    relu(0 + bias=1), manufacture ones-rows that fold the layer-2 biases
    into the matmuls.  fp16 operands keep each matmul a single PE pass.
  - PE: 6 accumulating L1 matmuls -> relu -> (dummy warm-up matmul keeps
    the PE pstate up through the relu gap) -> 2 L2 matmuls (fp16 w2 lhsT
    zero-padded to base partition 0); z's matmul goes first so tanh
    starts while mu's still runs.
  - Scalar: relu, tanh(scale=-1), exp (iv = exp(-tanh(z)); logvar
    cancels); exp's accumulator output emits A for free.
  - DVE: copies mu out of PSUM during its idle window, then 4 products +
    one segmented tensor_reduce -> C, Dv, U1v, U2v.  GpSimd up-casts yT.
  - No transposes, no collectives; one [128,7] output DMA whose
    completion overlaps the end-of-block drains.
"""

import numpy as np

import concourse.bacc as bacc
from concourse import mybir

F32 = mybir.dt.float32
FP16 = mybir.dt.float16
AF = mybir.ActivationFunctionType
ALU = mybir.AluOpType
AX = mybir.AxisListType

B, X_DIM, Y_DIM, HID = 512, 768, 128, 8
N_CORES = 8
R = B // N_CORES          # rows per core = 64
XC = X_DIM // 128         # x feature chunks = 6
WCOL = 18                 # packed L1 lhsT cols: 0:8 mu, 8 zero, 9:17 lv, 17 zero
WB = XC * WCOL + 1        # wp block cols = 109 (col 108 = b1vec)
MB = WB + XC * R + R      # mega blob cols = 109 + 384 + 64 = 557

_CACHE = {}


def _build():
    nc = bacc.Bacc("TRN2", target_bir_lowering=False, debug=False,
                   num_devices=N_CORES)

    mg_d = nc.dram_tensor("mg", [128, MB], FP16, kind="ExternalInput")
    # w2: cols 0:128 = mu block (rows 0:8 w2_mu, row 8 b2_mu, rows 9:18 zero)
    #     cols 128:256 = lv block (rows 0:9 zero, rows 9:17 w2_lv, row 17 b2_lv)
    w2_d = nc.dram_tensor("w2", [WCOL, 256], FP16, kind="ExternalInput")
    # out columns: A, C, Dv, U1v, U2v, S2, M1
    out_d = nc.dram_tensor("out", [128, 7], F32, kind="ExternalOutput")

    from contextlib import ExitStack
    with ExitStack() as stack:
        e = stack.enter_context
        mg_s = e(nc.sbuf_tensor([128, MB], FP16))
        w2_s = e(nc.sbuf_tensor([WCOL, 256], FP16))
        yt_s = e(nc.sbuf_tensor([128, R], F32))   # f32 up-cast of yT
        hb_s = e(nc.sbuf_tensor([WCOL, R], FP16))
        th_s = e(nc.sbuf_tensor([128, R], F32))
        iv_s = e(nc.sbuf_tensor([128, R], F32))
        mu_s = e(nc.sbuf_tensor([128, R], F32))
        prods = e(nc.sbuf_tensor([128, 4, R], F32))
        ysq_s = e(nc.sbuf_tensor([128, R], F32))
        outv = e(nc.sbuf_tensor([128, 7], F32))
        ps1 = e(nc.psum_tensor([WCOL, R], F32))
        psw = e(nc.psum_tensor([WCOL, R], F32))   # warm-up sink
        ps2l = e(nc.psum_tensor([128, R], F32))
        ps2m = e(nc.psum_tensor([128, R], F32))
        msem = e(nc.semaphore())    # mega blob landed
        w2sem = e(nc.semaphore())
        gsem = e(nc.semaphore())    # gpsimd: yt converted
        tsem = e(nc.semaphore())    # tensor progress: 1=L1, 2=ps2l, 3=ps2m
        ssem = e(nc.semaphore())    # scalar progress: 1=relu, 2=exp+A
        vsem = e(nc.semaphore())    # vector progress: 1=y-moments, 2=reduce
        osem = e(nc.semaphore())    # out DMA completion (not awaited)
        vq = e(nc.semaphore())      # vector same-engine chain
        sq = e(nc.semaphore())      # scalar same-engine chain
        block = e(nc.Block())

        yT = yt_s[:]
        xcol = WB              # xT chunks start col
        ycol = WB + XC * R     # yT start col

        @block.sync
        def _(sync):
            sync.dma_start(out=mg_s[:], in_=mg_d[:]).then_inc(msem, 16)
            sync.wait_ge(vsem, 2)
            sync.wait_ge(ssem, 2)
            sync.dma_start(out=out_d[:], in_=outv[:]).then_inc(osem, 16)

        @block.scalar
        def _(scalar):
            scalar.dma_start(out=w2_s[:], in_=w2_d[:]).then_inc(w2sem, 16)
            scalar.wait_ge(tsem, 1)
            scalar.activation(out=hb_s[:], in_=ps1[:], func=AF.Relu,
                              bias=mg_s[0:WCOL, WB - 1:WB]
                              ).then_inc(ssem, 1)
            scalar.wait_ge(tsem, 2)
            scalar.activation(out=th_s[:], in_=ps2l[:], func=AF.Tanh,
                              scale=-1.0).then_inc(sq, 1)
            scalar.wait_ge(sq, 1)
            scalar.activation(out=iv_s[:], in_=th_s[:], func=AF.Exp,
                              accum_out=outv[:, 0:1]).then_inc(ssem, 1)

        @block.tensor
        def _(tensor):
            tensor.wait_ge(msem, 16)
            for k in range(XC):
                inst = tensor.matmul(ps1[:],
                                     mg_s[:, k * WCOL:(k + 1) * WCOL],
                                     mg_s[:, xcol + k * R:xcol + (k + 1) * R],
                                     start=(k == 0), stop=(k == XC - 1))
                if k == XC - 1:
                    inst.then_inc(tsem, 1)
            # keep the PE pstate hot through the relu gap
            tensor.matmul(psw[:], mg_s[:, 0:WCOL],
                          mg_s[:, xcol:xcol + R], start=True, stop=True)
            tensor.wait_ge(ssem, 1)
            tensor.wait_ge(w2sem, 16)
            tensor.matmul(ps2l[:], w2_s[:, 128:256], hb_s[:],
                          start=True, stop=True).then_inc(tsem, 1)
            tensor.matmul(ps2m[:], w2_s[:, 0:128], hb_s[:],
                          start=True, stop=True).then_inc(tsem, 1)

        @block.gpsimd
        def _(gpsimd):
            gpsimd.wait_ge(msem, 16)
            gpsimd.tensor_copy(out=yt_s[:], in_=mg_s[:, ycol:ycol + R]
                               ).then_inc(gsem, 1)

        @block.vector
        def _(vector):
            vector.wait_ge(gsem, 1)
            vector.tensor_mul(ysq_s[:], yT, yT).then_inc(vq, 1)
            vector.wait_ge(vq, 1)
            vector.tensor_reduce(out=outv[:, 5:6], in_=ysq_s[:],
                                 axis=AX.X, op=ALU.add)
            vector.tensor_reduce(out=outv[:, 6:7], in_=yT,
                                 axis=AX.X, op=ALU.add).then_inc(vsem, 1)
            vector.wait_ge(tsem, 3)
            vector.tensor_copy(out=mu_s[:], in_=ps2m[:]).then_inc(vq, 1)
            vector.wait_ge(ssem, 2)
            vector.wait_ge(vq, 2)
            vector.tensor_mul(prods[:, 0, :], mu_s[:], iv_s[:]).then_inc(vq, 1)
            vector.wait_ge(vq, 3)
            vector.tensor_mul(prods[:, 1, :], prods[:, 0, :], mu_s[:]
                              ).then_inc(vq, 1)
            vector.tensor_mul(prods[:, 2, :], prods[:, 0, :], yT
                              ).then_inc(vq, 1)
            vector.tensor_mul(prods[:, 3, :], ysq_s[:], iv_s[:]
                              ).then_inc(vq, 1)
            vector.wait_ge(vq, 6)
            vector.tensor_reduce(out=outv[:, 1:5], in_=prods[:],
                                 axis=AX.X, op=ALU.add).then_inc(vsem, 1)

    nc.compile()
    return nc


def _get_nc():
    if "nc" not in _CACHE:
        _CACHE["nc"] = _build()
    return _CACHE["nc"]


def _pack(x_samples, y_samples, w1_mu, b1_mu, w2_mu, b2_mu,
          w1_lv, b1_lv, w2_lv, b2_lv):
    f = np.float32
    w1m = np.asarray(w1_mu, f).reshape(XC, 128, HID)
    w1l = np.asarray(w1_lv, f).reshape(XC, 128, HID)
    wp = np.zeros((128, WB), f)
    for k in range(XC):
        wp[:, k * WCOL:k * WCOL + 8] = w1m[k]
        wp[:, k * WCOL + 9:k * WCOL + 17] = w1l[k]
    wp[0:8, WB - 1] = np.asarray(b1_mu, f)
    wp[8, WB - 1] = 1.0
    wp[9:17, WB - 1] = np.asarray(b1_lv, f)
    wp[17, WB - 1] = 1.0

    w2 = np.zeros((WCOL, 256), f)
    w2[0:8, 0:128] = np.asarray(w2_mu, f)
    w2[8, 0:128] = np.asarray(b2_mu, f)
    w2[9:17, 128:256] = np.asarray(w2_lv, f)
    w2[17, 128:256] = np.asarray(b2_lv, f)
    w2 = w2.astype(np.float16)

    x = np.asarray(x_samples, f)
    y = np.asarray(y_samples, f)
    in_maps = []
    for c in range(N_CORES):
        xs = x[c * R:(c + 1) * R]                       # [64, 768]
        xT = xs.reshape(R, XC, 128).transpose(1, 2, 0)  # [6, 128, 64]
        mg = np.empty((128, MB), np.float16)
        mg[:, 0:WB] = wp
        mg[:, WB:WB + XC * R] = xT.transpose(1, 0, 2).reshape(128, XC * R)
        mg[:, WB + XC * R:MB] = y[c * R:(c + 1) * R].T
        in_maps.append({"mg": mg, "w2": w2})
    return in_maps


def kernel(x_samples, y_samples, w1_mu, b1_mu, w2_mu, b2_mu,
           w1_lv, b1_lv, w2_lv, b2_lv, **profile_kwargs):
    from concourse import bass_utils

    in_maps = _pack(x_samples, y_samples, w1_mu, b1_mu, w2_mu, b2_mu,
                    w1_lv, b1_lv, w2_lv, b2_lv)
    nc = _get_nc()
    res = bass_utils.run_bass_kernel_spmd(
        nc, in_maps, core_ids=list(range(N_CORES)), **profile_kwargs
    )
    acc = np.zeros((128, 7), np.float64)
    for m in res.results:
        acc += m["out"].astype(np.float64)
    A, C, Dv, U1v, U2v, S2, M1 = (acc[:, j] for j in range(7))
    D = Dv.sum()
    P = D - 2.0 * U1v.sum() + U2v.sum()
    neg = (S2 @ A - 2.0 * (M1 @ C) + B * D) / (2.0 * B * B)
    total = -P / (2.0 * B) + neg - np.log1p(np.exp(-20.0) / (B - 1.0))
    out = np.array(total, dtype=np.float32)
    if profile_kwargs:
        return out, res
    return out
